# revision 16
# baseline (speedup 1.0000x reference)
"""Trainium2 Bass kernel for nn_AttentionCell (additive attention + GRU cell).

Full-input contract: kernel(**inputs) takes the unsharded inputs, shards the
batch dim (512) across 8 NeuronCores (64 rows each), runs an SPMD Bass/Tile
program, and reassembles the full outputs (cur_hidden [512,512], alpha.T
[240,512]).

Per-core dataflow (B=64, C=512, HW=240, E=128):
  phase A: stream conv_feats in C-on-partitions layout;
           e_pre = cf + h (DVE tensor_scalar, bf16 out), e = tanh (ACT),
           emition[b,h] = sum_c w[c]*e[c,h] via PE matmuls (w column
           stationary), rows gathered on partition 0 and bounced via DRAM.
  phase B: softmax over HW on [64,240] (DVE reduce + ACT exp w/ accum + DVE).
  phase C: stream conv_feats_origin in (c-half,batch)-on-partitions layout;
           context = DVE multiply by broadcast alpha + ACT Identity
           accum_out reduce over HW.
  phase D: GRU cell as two augmented matmuls (bias folded in via a ones row;
           output in [batch, 3C] layout), gates on ACT/DVE, outputs DMA'd.
"""

import sys

if "/opt/trn_rl_repo" not in sys.path:
    sys.path.insert(0, "/opt/trn_rl_repo")

import numpy as np
import ml_dtypes

import concourse.bass as bass  # noqa: F401
import concourse.tile as tile
from concourse import bacc, mybir
from concourse.bass_utils import run_bass_kernel_spmd
from concourse.masks import make_identity

# problem constants (hardcoded; harness provides no sibling files)
NB, C, H, W, E = 512, 512, 6, 40, 128
HW = H * W  # 240
NCORES = 8
B = NB // NCORES  # 64 rows per core
CT = C // 128     # 4 c-tiles in c-partition layout
J = 2             # c halves in (j,b)-partition layout
CH = C // J       # 256 c's per half
BC = 8            # batch rows per conv_feats chunk   -> 8 chunks, 4 DMAs of ~1MB
CC = 16           # c' columns per conv_feats_origin chunk -> 16 chunks, 2 DMAs of ~1MB

F32 = mybir.dt.float32
BF16 = mybir.dt.bfloat16
AF = mybir.ActivationFunctionType
ALU = mybir.AluOpType
AX = mybir.AxisListType


def build_nc(reps: int = 1):
    nc = bacc.Bacc("TRN2", target_bir_lowering=False, debug=False)

    ph_d = nc.dram_tensor("ph", [B, C], F32, kind="ExternalInput").ap()
    cf_d = nc.dram_tensor("cf", [B, C, HW], F32, kind="ExternalInput").ap()
    cfo_d = nc.dram_tensor("cfo", [B, C, HW], F32, kind="ExternalInput").ap()
    emb_d = nc.dram_tensor("emb", [B, E], F32, kind="ExternalInput").ap()
    swt_d = nc.dram_tensor("swt", [128, CT], BF16, kind="ExternalInput").ap()
    # w1 = [w_ih.T ; b_ih] (641 x 1536), w2 = [w_hh.T ; b_hh] (513 x 1536)
    w1_d = nc.dram_tensor("w1", [C + E + 1, 3 * C], BF16, kind="ExternalInput").ap()
    w2_d = nc.dram_tensor("w2", [C + 1, 3 * C], BF16, kind="ExternalInput").ap()

    hid_d = nc.dram_tensor("hid", [B, C], F32, kind="ExternalOutput").ap()
    alp_d = nc.dram_tensor("alp", [HW, B], F32, kind="ExternalOutput").ap()
    em_bounce = nc.dram_tensor("em_bounce", [B, HW], F32).ap()
    em_bv = em_bounce.rearrange("b h -> (b h)").rearrange(
        "(q o b h) -> q o b h", o=1, b=BC, h=HW
    )  # [B//BC, 1, BC, HW]

    cf_r = cf_d.rearrange("b (t p) h -> p t b h", p=128)  # [128,4,64,240]
    cfo_r = cfo_d.rearrange("b (j c) h -> j b c h", j=J)  # [2,64,256,240]

    with tile.TileContext(nc) as tc:
        with (
            tc.tile_pool(name="const", bufs=1) as cp,
            tc.tile_pool(name="tp_ps", bufs=2, space="PSUM") as tpp,
            tc.tile_pool(name="em_ps", bufs=2, space="PSUM") as emp,
            tc.tile_pool(name="gru_ps", bufs=1, space="PSUM") as gp,
            tc.tile_pool(name="cfp", bufs=2) as cfp,
            tc.tile_pool(name="epool", bufs=2) as epool,
            tc.tile_pool(name="cfop", bufs=2) as cfop,
            tc.tile_pool(name="scr", bufs=2) as scrp,
        ):

            def body():
                ident = cp.tile([128, 128], F32, tag="ident")
                make_identity(nc, ident[:])

                swt = cp.tile([128, CT], BF16, tag="swt")
                nc.sync.dma_start(out=swt[:], in_=swt_d[:])

                ph_sb = cp.tile([B, C], F32, tag="ph_sb")
                nc.sync.dma_start(out=ph_sb[:], in_=ph_d[:])
                emb_sb = cp.tile([B, E], F32, tag="emb_sb")
                nc.sync.dma_start(out=emb_sb[:], in_=emb_d[:])

                # prev_hidden transposed to c-on-partitions, f32 + bf16 copies
                hT_f, hT_b = [], []
                for t in range(CT):
                    ps = tpp.tile([128, B], F32, tag="tp")
                    nc.tensor.transpose(
                        ps[:], ph_sb[:, t * 128 : (t + 1) * 128], ident[:B, :B]
                    )
                    f_ = cp.tile([128, B], F32, tag=f"hTf{t}")
                    nc.vector.tensor_copy(out=f_[:], in_=ps[:])
                    b_ = cp.tile([128, B], BF16, tag=f"hTb{t}")
                    nc.scalar.copy(out=b_[:], in_=ps[:])
                    hT_f.append(f_)
                    hT_b.append(b_)

                ps = tpp.tile([128, B], F32, tag="tp")
                nc.tensor.transpose(ps[:], emb_sb[:, :], ident[:B, :B])
                embT = cp.tile([128, B], BF16, tag="embT")
                nc.vector.tensor_copy(out=embT[:], in_=ps[:])

                ones_row = cp.tile([1, B], BF16, tag="ones_row")
                nc.vector.memset(ones_row[:], 1.0)

                # --- phase A: conv_feats -> e -> emition rows on partition 0
                em_sb = cp.tile([B, HW], F32, tag="em_sb")
                for q in range(B // BC):
                    cf_sb = cfp.tile([128, CT, BC, HW], F32, tag="cf")
                    for t in range(CT):
                        nc.sync.dma_start(
                            out=cf_sb[:, t], in_=cf_r[:, t, q * BC : (q + 1) * BC, :]
                        )
                    e_sb = epool.tile([128, CT, BC, HW], BF16, tag="e")
                    for t in range(CT):
                        for bi in range(BC):
                            b0 = q * BC + bi
                            nc.vector.tensor_scalar_add(
                                out=e_sb[:, t, bi, :],
                                in0=cf_sb[:, t, bi, :],
                                scalar1=hT_f[t][:, b0 : b0 + 1],
                            )
                    for t in range(CT):
                        nc.scalar.activation(
                            out=e_sb[:, t], in_=e_sb[:, t], func=AF.Tanh
                        )
                    em_flat = scrp.tile([1, BC, HW], F32, tag="em_flat")
                    for bi in range(BC):
                        emr = emp.tile([1, HW], F32, tag="em")
                        for t in range(CT):
                            nc.tensor.matmul(
                                emr[:],
                                lhsT=swt[:, t : t + 1],
                                rhs=e_sb[:, t, bi, :],
                                start=(t == 0),
                                stop=(t == CT - 1),
                            )
                        nc.scalar.copy(out=em_flat[:, bi, :], in_=emr[:])
                    nc.sync.dma_start(out=em_bv[q], in_=em_flat[0:1])

                nc.sync.dma_start(out=em_sb[:], in_=em_bounce[:])

                # --- phase B: softmax over HW
                neg_mx = cp.tile([B, 1], F32, tag="neg_mx")
                nc.vector.tensor_reduce(
                    out=neg_mx[:], in_=em_sb[:], axis=AX.X, op=ALU.max, negate=True
                )
                exp_sb = cp.tile([B, HW], F32, tag="exp_sb")
                ssum = cp.tile([B, 1], F32, tag="ssum")
                nc.scalar.activation(
                    out=exp_sb[:],
                    in_=em_sb[:],
                    func=AF.Exp,
                    bias=neg_mx[:],
                    accum_out=ssum[:],
                )
                rinv = cp.tile([B, 1], F32, tag="rinv")
                nc.vector.reciprocal(rinv[:], ssum[:])
                alpha_sb = cp.tile([B, HW], F32, tag="alpha_sb")
                nc.vector.tensor_scalar_mul(
                    out=alpha_sb[:], in0=exp_sb[:], scalar1=rinv[:]
                )
                alpha2 = cp.tile([128, HW], F32, tag="alpha2")
                nc.vector.tensor_copy(out=alpha2[0:B, :], in_=alpha_sb[:])
                nc.vector.tensor_copy(out=alpha2[B:128, :], in_=alpha_sb[:])

                # alpha.T output [240, 64]
                aps1 = tpp.tile([128, B], F32, tag="tp")
                nc.tensor.transpose(aps1[:], alpha_sb[:, 0:128], ident[:B, :B])
                at1 = cp.tile([128, B], F32, tag="at1")
                nc.vector.tensor_copy(out=at1[:], in_=aps1[:])
                nc.sync.dma_start(out=alp_d[0:128, :], in_=at1[:])
                aps2 = tpp.tile([HW - 128, B], F32, tag="tp")
                nc.tensor.transpose(aps2[:], alpha_sb[:, 128:HW], ident[:B, :B])
                at2 = cp.tile([HW - 128, B], F32, tag="at2")
                nc.vector.tensor_copy(out=at2[:], in_=aps2[:])
                nc.sync.dma_start(out=alp_d[128:HW, :], in_=at2[:])

                # GRU weights: DMA on the ACT HWDGE ring, overlapping phase C
                w1t = []
                for t in range(5):
                    wt_ = cp.tile([128, 3 * C], BF16, tag=f"w1_{t}")
                    nc.scalar.dma_start(
                        out=wt_[:], in_=w1_d[t * 128 : (t + 1) * 128, :]
                    )
                    w1t.append(wt_)
                w1o = cp.tile([1, 3 * C], BF16, tag="w1o")
                nc.scalar.dma_start(out=w1o[:], in_=w1_d[C + E : C + E + 1, :])
                w2t = []
                for t in range(4):
                    wt_ = cp.tile([128, 3 * C], BF16, tag=f"w2_{t}")
                    nc.scalar.dma_start(
                        out=wt_[:], in_=w2_d[t * 128 : (t + 1) * 128, :]
                    )
                    w2t.append(wt_)
                w2o = cp.tile([1, 3 * C], BF16, tag="w2o")
                nc.scalar.dma_start(out=w2o[:], in_=w2_d[C : C + 1, :])

                # --- phase C: conv_feats_origin -> context
                ctx_sb = cp.tile([128, CH], F32, tag="ctx_sb")
                for cc in range(CH // CC):
                    cfo_sb = cfop.tile([128, CC, HW], F32, tag="cfo")
                    for j in range(J):
                        nc.sync.dma_start(
                            out=cfo_sb[j * B : (j + 1) * B],
                            in_=cfo_r[j, :, cc * CC : (cc + 1) * CC, :],
                        )
                    for ci in range(CC):
                        c0 = cc * CC + ci
                        # tensor_tensor_reduce hangs TRN2 here; use DVE mult +
                        # ACT Identity reduce (accum_out) instead.
                        scr = scrp.tile([128, HW], F32, tag="scr")
                        nc.vector.tensor_tensor(
                            out=scr[:], in0=cfo_sb[:, ci, :], in1=alpha2[:], op=ALU.mult
                        )
                        nc.scalar.activation(
                            out=scr[:],
                            in_=scr[:],
                            func=AF.Identity,
                            accum_out=ctx_sb[:, c0 : c0 + 1],
                        )

                # --- phase D: GRU cell
                xTb = []
                for blk in range(2):
                    psx = tpp.tile([128, 128], F32, tag="tp")
                    nc.tensor.transpose(
                        psx[:], ctx_sb[:, blk * 128 : (blk + 1) * 128], ident[:]
                    )
                    xb = cp.tile([128, 128], BF16, tag=f"xT{blk}")
                    nc.vector.tensor_copy(out=xb[:], in_=psx[:])
                    xTb.append(xb)
                x_lhs = [
                    xTb[0][:, 0:B],    # c 0:128   (j=0, blk=0)
                    xTb[1][:, 0:B],    # c 128:256 (j=0, blk=1)
                    xTb[0][:, B:128],  # c 256:384 (j=1, blk=0)
                    xTb[1][:, B:128],  # c 384:512 (j=1, blk=1)
                    embT[:],           # e 0:128
                ]
                h_lhs = [hT_b[t][:] for t in range(CT)]

                rz_ps = gp.tile([B, 2, 512], F32, tag="rz")
                xn_ps = gp.tile([B, 512], F32, tag="xn")
                hn_ps = gp.tile([B, 512], F32, tag="hn")

                seq = (
                    [(x_lhs[i], w1t[i]) for i in range(5)]
                    + [(ones_row[:], w1o)]
                    + [(h_lhs[i], w2t[i]) for i in range(4)]
                    + [(ones_row[:], w2o)]
                )
                last = len(seq) - 1
                for idx, (lh, wsb) in enumerate(seq):
                    nc.tensor.matmul(
                        rz_ps[:, 0], lhsT=lh, rhs=wsb[:, 0:512],
                        start=(idx == 0), stop=(idx == last),
                    )
                    nc.tensor.matmul(
                        rz_ps[:, 1], lhsT=lh, rhs=wsb[:, 512:1024],
                        start=(idx == 0), stop=(idx == last),
                    )
                    tgt = xn_ps if idx < 6 else hn_ps
                    nc.tensor.matmul(
                        tgt[:], lhsT=lh, rhs=wsb[:, 1024:1536],
                        start=(idx in (0, 6)), stop=(idx in (5, last)),
                    )

                r_sb = cp.tile([B, 512], F32, tag="r_sb")
                nc.scalar.activation(out=r_sb[:], in_=rz_ps[:, 0], func=AF.Sigmoid)
                z_sb = cp.tile([B, 512], F32, tag="z_sb")
                nc.scalar.activation(out=z_sb[:], in_=rz_ps[:, 1], func=AF.Sigmoid)
                t2 = cp.tile([B, 512], F32, tag="t2")
                nc.vector.tensor_tensor(out=t2[:], in0=r_sb[:], in1=hn_ps[:], op=ALU.mult)
                nc.vector.tensor_tensor(out=t2[:], in0=t2[:], in1=xn_ps[:], op=ALU.add)
                n_sb = cp.tile([B, 512], F32, tag="n_sb")
                nc.scalar.activation(out=n_sb[:], in_=t2[:], func=AF.Tanh)
                d_sb = cp.tile([B, 512], F32, tag="d_sb")
                nc.vector.tensor_tensor(
                    out=d_sb[:], in0=ph_sb[:], in1=n_sb[:], op=ALU.subtract
                )
                nc.vector.tensor_tensor(out=d_sb[:], in0=z_sb[:], in1=d_sb[:], op=ALU.mult)
                nc.vector.tensor_tensor(out=d_sb[:], in0=n_sb[:], in1=d_sb[:], op=ALU.add)
                nc.sync.dma_start(out=hid_d[:], in_=d_sb[:])

            if reps == 1:
                body()
            else:
                engs = (
                    mybir.EngineType.PE,
                    mybir.EngineType.DVE,
                    mybir.EngineType.Activation,
                    mybir.EngineType.SP,
                    mybir.EngineType.Pool,
                )
                with tc.For_i(0, reps, 1, hint_engines=engs):
                    body()

    nc.compile()
    return nc


_NC_CACHE = None


def _get_nc():
    global _NC_CACHE
    if _NC_CACHE is None:
        _NC_CACHE = build_nc()
    return _NC_CACHE


def make_in_maps(inputs):
    ph = np.ascontiguousarray(np.asarray(inputs["prev_hidden"], np.float32))
    cf = np.ascontiguousarray(np.asarray(inputs["conv_feats"], np.float32).reshape(NB, C, HW))
    cfo = np.ascontiguousarray(
        np.asarray(inputs["conv_feats_origin"], np.float32).reshape(NB, C, HW)
    )
    emb = np.ascontiguousarray(np.asarray(inputs["cur_embeddings"], np.float32))
    sw = np.asarray(inputs["score_w"], np.float32)
    w_ih = np.asarray(inputs["w_ih"], np.float32)
    w_hh = np.asarray(inputs["w_hh"], np.float32)
    b_ih = np.asarray(inputs["b_ih"], np.float32)
    b_hh = np.asarray(inputs["b_hh"], np.float32)

    swt = np.ascontiguousarray(sw.reshape(CT, 128).T).astype(ml_dtypes.bfloat16)
    w1 = np.ascontiguousarray(np.vstack([w_ih.T, b_ih[None, :]])).astype(ml_dtypes.bfloat16)
    w2 = np.ascontiguousarray(np.vstack([w_hh.T, b_hh[None, :]])).astype(ml_dtypes.bfloat16)

    in_maps = []
    for i in range(NCORES):
        sl = slice(i * B, (i + 1) * B)
        in_maps.append(
            dict(
                ph=np.ascontiguousarray(ph[sl]),
                cf=np.ascontiguousarray(cf[sl]),
                cfo=np.ascontiguousarray(cfo[sl]),
                emb=np.ascontiguousarray(emb[sl]),
                swt=swt,
                w1=w1,
                w2=w2,
            )
        )
    return in_maps


def kernel(**inputs):
    nc = _get_nc()
    in_maps = make_in_maps(inputs)
    res = run_bass_kernel_spmd(nc, in_maps, list(range(NCORES))).results
    hid = np.concatenate([res[i]["hid"] for i in range(NCORES)], axis=0)
    alp = np.concatenate([res[i]["alp"] for i in range(NCORES)], axis=1)
    return hid.astype(np.float32), alp.astype(np.float32)
